# revision 39
# baseline (speedup 1.0000x reference)
"""DenseEdgeGAT layer on 8 trn2 NeuronCores (Bass/Tile).

Strategy (row-sharding over target nodes n, 128 rows per core):
  - Every core computes LN(x) + k/v for all 1024 source nodes (replicated,
    cheap) and q for its own 128-row slab.
  - Host-side algebra shortens the on-device critical path:
      * gamma/beta of the pre-LN fold into Wq/Wk/Wv (and their biases),
        so LN on device is just (x - mu) * rstd.
      * bk drops entirely: q . bk is constant over source nodes m, and
        softmax over m is invariant to per-(n,h) shifts.
      * bv folds into bo (sum_m alpha = 1), so v needs no bias pass.
      * the 0.25 attention scale folds into Wq.
  - edge_feat slab is uploaded twice with different layouts:
      * n-major fp16 -> X[(m%8,e), j, n] feeding the block-diagonal Wae
        matmuls (edge attention bias).  The -10000 attention mask is
        folded into these values on the host: ef' = ef + (1-mask) * v0
        where Wae^T v0 = -10000 * ones, so no separate mask pass exists.
      * m-major fp8-e4m3 -> Z[m%128, mc, n, e32pad] feeding the per-node
        T-stage (edge-value contraction): T[n,h,e] = sum_m p[n,m,h]
        ef[n,m,e], out2 = T @ Wve; 4 nodes per matmul, diagonal blocks at
        32-aligned partition bases (hardware requires 32-aligned
        partition access).  fp8 value-path rounding costs ~5e-3 absmax
        rel err (measured vs fp32 reference), under the 2e-2 gate.
  - softmax uses a fixed shift M=50 (shift-invariant, no rowmax pass);
    bae - M enters as the scalar operand of the single score-merge op.
    Bias is staged head-major [n, (h, m)] so the merge reads contiguous
    fp16; staging copies run on the scalar engine (idle in that phase).
    exp runs on the scalar engine with fused row-sum accumulation; 1/denom
    is applied once at the end (flash-attention style).
  - p^T for the T-stage/out1 comes from PE transposes (SBUF->SBUF DMA
    transposes were tried and are ~10x slower: 165 tiny descriptors per
    [128,128] tile swamp the queues).
  - DMA: x/wblk go first on the sync queue so LN/bias can start ASAP,
    then efx in 4 chunks; Z follows efx; params ride the gpsimd queue in
    3 consolidated transfers (each dma_start costs ~0.9us of issue time).
"""

import numpy as np
import ml_dtypes

import concourse.bacc as bacc
import concourse.bass as bass
import concourse.tile as tile
from concourse import mybir
from concourse.bass_utils import run_bass_kernel_spmd
from concourse.masks import make_identity

F32 = mybir.dt.float32
F32R = mybir.dt.float32r
F16 = mybir.dt.float16
BF16 = mybir.dt.bfloat16
F8 = mybir.dt.float8e4

N = 1024
DIM = 128
H = 8
DK = 16
E = 16
P = 128
NCORES = 8
R = N // NCORES  # 128 rows per core
NMC = N // P     # 8 m-chunks
SHIFT = 50.0
EPS = 1e-5
NEG = -10000.0

AF = mybir.ActivationFunctionType
OP = mybir.AluOpType

_CACHED = None


def _build_program(dbg=False):
    nc = bacc.Bacc("TRN2", target_bir_lowering=False, debug=False,
                   num_devices=NCORES)

    dram = {}

    def din(name, shape, dt):
        dram[name] = nc.dram_tensor(name, shape, dt, kind="ExternalInput").ap()
        return dram[name]

    efx_d = din("efx", [P, 128 * P], F16)      # X[(c,e), (j, n)] host-transposed
    efm_d = din("efm", [N, R * 32], F8)        # m-major slab [m, (n, e32pad)]
    xarr_d = din("xarr", [P, (NMC + 1) * DIM], F32)  # x pre-arranged + slab
    wbig_d = din("wbig", [DIM, 8 * DIM], F32)  # [Wq_p | Wk_p | Wv' | Wo]
    wve_d = din("wve", [E, DIM], F32)
    wblk_d = din("wblk", [P, 8 * H], F16)      # block-diag Wae, cols (h, c)
    smalls_d = din("smalls", [P, 3 + H], F32)        # bq_p | bae-M
    boxs_d = din("boxs", [R, DIM], F32)        # bo' + x slab (residual)

    out_d = nc.dram_tensor("out", [R, DIM], F32, kind="ExternalOutput").ap()

    with tile.TileContext(nc) as tc:
        with tc.tile_pool(name="consts", bufs=1) as consts, \
             tc.tile_pool(name="big", bufs=1) as big, \
             tc.tile_pool(name="attring", bufs=2) as attring, \
             tc.tile_pool(name="work", bufs=2) as work, \
             tc.tile_pool(name="praws", bufs=1) as praws, \
             tc.tile_pool(name="psbig", bufs=2, space="PSUM") as psbig, \
             tc.tile_pool(name="pssm", bufs=2, space="PSUM") as pssm:

            # ---------- constants / params ----------
            ident = consts.tile([P, P], F32)
            make_identity(nc, ident)
            identb = consts.tile([P, P], BF16)
            make_identity(nc, identb)
            # preload the EXP table first thing (exp is the only ACT
            # function used; Copy needs no table) and keep the PE busy
            # through the DMA wait so the HAM clock gate stays open.
            dummy_o = consts.tile([P, 1], F32)
            # ~9.5us of warmup keeps the PE's HAM clock gate open until the
            # first efx chunk lands (cold PE at 1.2GHz was stretching the
            # whole bias/projection phase by ~40%)
            warm = psbig.tile([P, P], F32, tag="pb")
            for w in range(88):
                nc.tensor.transpose(warm, ident, ident)


            # critical-path uploads first on the sync queue: x, wblk
            # (x is pre-arranged on the host so this is one contiguous
            # transfer -- a partition-scatter AP here ran at 125 GB/s)
            xall = big.tile([P, NMC + 1, DIM], F32)
            nc.sync.dma_start(out=xall, in_=xarr_d
                              .rearrange("p (g d) -> p g d", d=DIM))
            wblk_t = consts.tile([P, 8 * H], F16)
            nc.sync.dma_start(out=wblk_t, in_=wblk_d)

            # consolidated params on the gpsimd queue
            smalls_t = consts.tile([P, 3 + H], F32)
            nc.scalar.dma_start(out=smalls_t, in_=smalls_d)
            bq_t = smalls_t[:, 0:3]
            baerep_t = smalls_t[:, 3:3 + H]
            boxs_t = consts.tile([R, DIM], F32)
            nc.scalar.dma_start(out=boxs_t, in_=boxs_d)

            wbig_raw = praws.tile([DIM, 8 * DIM], F32, tag="praw_wbig")
            nc.scalar.dma_start(out=wbig_raw, in_=wbig_d)
            wbig_r = consts.tile([DIM, 8 * DIM], F32R, tag="wbig")
            nc.vector.tensor_copy(wbig_r, wbig_raw)
            wq_r = wbig_r[:, 0:3 * DIM]
            wk_r = wbig_r[:, 3 * DIM:6 * DIM]
            wv_r = wbig_r[:, 6 * DIM:7 * DIM]
            wo_r = wbig_r[:, 7 * DIM:8 * DIM]

            wve_raw = praws.tile([E, DIM], F32, tag="praw_wve")
            nc.scalar.dma_start(out=wve_raw, in_=wve_d)
            wve_r = consts.tile([E, DIM], F32R, tag="wve")
            nc.vector.tensor_copy(wve_r, wve_raw)

            # ---------- big persistent tensors ----------
            # X on the sync HWDGE queue in 4 chunks (bias matmuls start as
            # soon as the first chunk lands); Z on the scalar HWDGE queue.
            x_full = big.tile([P, 128, P], F16)        # X[(c,e), j, n]
            for q in range(4):
                nc.sync.dma_start(
                    out=x_full[:, q * 32:(q + 1) * 32, :],
                    in_=efx_d[:, q * 4096:(q + 1) * 4096]
                    .rearrange("p (j n) -> p j n", n=P))
            # Z rides the same sync queue AFTER efx: per-queue FIFO gives
            # efx full bandwidth first, no cross-engine dep needed.
            z_sb = big.tile([P, NMC, R, 32], F8)       # Z[mm, mc, n, e32]
            nc.sync.dma_start(
                out=z_sb,
                in_=efm_d.rearrange("(a b) c -> b a c", b=P))

            bias_sb = big.tile([R, H, N], F16)         # [n, h, m] bias (+mask)
            p_sb = big.tile([R, H, N], BF16)           # exp'd attention [n, h, m]
            pt_sb = big.tile([P, H, NMC, R], BF16)     # p transposed [mm, h, mc, n]
            kt_sb = big.tile([DIM, 3, N], BF16)        # k^T head-padded planes
            qts_sb = big.tile([DIM, 3, R], BF16)       # q^T slab, padded, pre-scaled
            ht_sb = big.tile([DIM, N], F32R)           # h^T [din, m]
            v_sb = big.tile([P, NMC, DIM], BF16)       # v [mm, mc, dout]
            den_t = big.tile([R, H], F32)              # softmax denominators

            # ---------- LN over all nodes (+ slab as chunk 8) ----------
            # gamma/beta folded into the projection weights on the host,
            # so h = (x - mu) * rstd here.  One-pass mean/var via bn_stats.
            hall = big.tile([P, NMC + 1, DIM], F32)
            x2 = big.tile([P, NMC + 1, DIM], F32)
            nc.gpsimd.tensor_tensor(out=x2, in0=xall, in1=xall, op=OP.mult)
            s1 = work.tile([P, NMC + 1], F32, tag="lnstat")
            nc.vector.tensor_reduce(out=s1, in_=xall, axis=mybir.AxisListType.X,
                                    op=OP.add)
            mu = work.tile([P, NMC + 1], F32, tag="lnmu")
            nc.vector.tensor_scalar_mul(mu, s1, 1.0 / DIM)
            s2 = work.tile([P, NMC + 1], F32, tag="lnstat")
            nc.vector.tensor_reduce(out=s2, in_=x2, axis=mybir.AxisListType.X,
                                    op=OP.add)
            mu2 = work.tile([P, NMC + 1], F32, tag="lnstat")
            nc.vector.tensor_tensor(out=mu2, in0=mu, in1=mu, op=OP.mult)
            var = work.tile([P, NMC + 1], F32, tag="lnvar")
            nc.vector.scalar_tensor_tensor(
                out=var, in0=s2, scalar=1.0 / DIM, op0=OP.mult,
                in1=mu2, op1=OP.subtract)
            eps_t = consts.tile([P, 1], F32)
            nc.vector.memset(eps_t, EPS)
            sd = work.tile([P, NMC + 1], F32, tag="lnsd")
            nc.scalar.activation(out=sd, in_=var, func=AF.Sqrt, bias=eps_t)
            # preload the EXP table right behind the sqrt (Copy, used by the
            # bias staging, needs no table; exp then never reloads)
            nc.scalar.activation(out=dummy_o, in_=sd[:, 0:1], func=AF.Exp)
            rstd = work.tile([P, NMC + 1], F32, tag="lnrstd")
            nc.vector.reciprocal(rstd, sd)
            shift_t = consts.tile([P, 1], F32)
            nc.vector.memset(shift_t, -SHIFT)
            for g in range(NMC + 1):
                nc.vector.scalar_tensor_tensor(
                    out=hall[:, g, :], in0=xall[:, g, :],
                    scalar=mu[:, g:g + 1], op0=OP.subtract,
                    in1=rstd[:, g:g + 1].broadcast_to([P, DIM]),
                    op1=OP.mult)

            # ---------- edge bias, first half (efx chunks 0-1) --------------
            # b_ps cols per j are (h, c); copy to head-major bias_sb[n, h, m]
            # (m = (mc*16+j)*8+c) so the score merge reads contiguous fp16.
            def bias_chunk(mc):
                b_ps = psbig.tile([R, 16 * 8 * H], F32, tag="pb")
                for xloc in range(16):
                    j = mc * 16 + xloc
                    nc.tensor.matmul(b_ps[:, xloc * 64:(xloc + 1) * 64],
                                     x_full[:, j, :], wblk_t,
                                     start=True, stop=True)
                nc.scalar.copy(
                    bias_sb[:, :, mc * P:(mc + 1) * P]
                        .rearrange("p h (j c) -> p h j c", c=8),
                    b_ps.rearrange("p (j h c) -> p h j c", h=H, c=8))

            for mc in range(4):
                bias_chunk(mc)

            # ---------- h^T via PE transposes ----------
            hst_sb = big.tile([DIM, R], F32R)  # h^T of the slab
            for g in range(NMC + 1):
                tp = pssm.tile([P, P], F32, tag="ps")
                nc.tensor.transpose(tp, hall[:, g, :], ident)
                if g < NMC:
                    nc.vector.tensor_copy(ht_sb[:, g * P:(g + 1) * P], tp)
                else:
                    nc.vector.tensor_copy(hst_sb, tp)

            # ---------- projections ----------
            # k^T = Wk^T-form: lhsT=Wk_pad plane, rhs=h^T [din, m]; no bias
            # (bk is softmax-invariant).
            for s in range(3):
                for i in range(2):
                    kp = psbig.tile([DIM, 512], F32, tag="pb")
                    nc.tensor.matmul(kp, wk_r[:, s * DIM:(s + 1) * DIM],
                                     ht_sb[:, i * 512:(i + 1) * 512],
                                     start=True, stop=True)
                    nc.vector.tensor_copy(kt_sb[:, s, i * 512:(i + 1) * 512],
                                          kp)
                qp = pssm.tile([DIM, R], F32, tag="ps")
                nc.tensor.matmul(qp, wq_r[:, s * DIM:(s + 1) * DIM], hst_sb,
                                 start=True, stop=True)
                nc.vector.tensor_scalar(out=qts_sb[:, s, :], in0=qp,
                                        scalar1=bq_t[:, s:s + 1], scalar2=None,
                                        op0=OP.add)

            # ---------- edge bias, second half (efx chunks 2-3) -------------
            for mc in range(4, NMC):
                bias_chunk(mc)

            # v natural: per chunk, lhsT=h^T chunk [din, m128], rhs=Wv
            for mc in range(NMC):
                vp = pssm.tile([P, DIM], F32, tag="ps")
                nc.tensor.matmul(vp, ht_sb[:, mc * P:(mc + 1) * P], wv_r,
                                 start=True, stop=True)
                nc.vector.tensor_copy(v_sb[:, mc, :], vp)

            # ---------- scores + merge + exp per head; then p^T + out1 ------
            # out1 accumulates directly into one PSUM tile.  Each head's
            # start=True clears has_written for the whole bank, but earlier
            # heads' columns are final data by then (PE is in-order) and
            # nothing accumulates onto them afterwards (out2 uses its own
            # tile), so the data survives.
            o1_ps = pssm.tile([R, DIM], F32, tag="o1h", bufs=1)
            for h in range(H):
                s, g = h // 3, h % 3
                sc_ps = psbig.tile([R, N], F32, tag="pb")
                for i in range(2):
                    nc.tensor.matmul(sc_ps[:, i * 512:(i + 1) * 512],
                                     qts_sb[g * 32:(g + 1) * 32, s, :],
                                     kt_sb[g * 32:(g + 1) * 32, s,
                                           i * 512:(i + 1) * 512],
                                     start=True, stop=True)
                att_t = attring.tile([R, N], F16)
                nc.vector.scalar_tensor_tensor(
                    out=att_t, in0=sc_ps, scalar=baerep_t[:, h:h + 1],
                    in1=bias_sb[:, h, :], op0=OP.add, op1=OP.add)
                nc.scalar.activation(out=p_sb[:, h, :], in_=att_t, func=AF.Exp,
                                     bias=shift_t,
                                     accum_out=den_t[:, h:h + 1])
                # p^T for this head (PE transposes, 4 chunks per copy), then
                # out1 partial sums run during the next head's exp.
                ptp = pssm.tile([P, NMC, P], BF16, tag="ps")
                for mc in range(NMC):
                    nc.tensor.transpose(ptp[:, mc, :],
                                        p_sb[:, h, mc * P:(mc + 1) * P],
                                        identb)
                nc.vector.tensor_copy(pt_sb[:, h, :, :], ptp)
                for mc in range(NMC):
                    nc.tensor.matmul(o1_ps[:, h * DK:(h + 1) * DK],
                                     pt_sb[:, h, mc, :],
                                     v_sb[:, mc, h * DK:(h + 1) * DK],
                                     start=(mc == 0), stop=(mc == NMC - 1))

            # ---------- T-stage: batched over 4 nodes per matmul ------------
            # lhsT = Z slice [mm, (4 nl x 32 e-pad)], rhs = pT [mm, (h, 4nl')]
            # -> out [(nl, e32), (h, nl')]; diagonal blocks at 32-aligned
            # partition bases; e rows 16..32 are pad (ignored).
            # ---------- out = (o1 + o2)/den @ Wo + box, split so the o1
            # half of the projection runs during the T-stage ------------
            rden = work.tile([R, H], F32, tag="rden")
            nc.vector.reciprocal(rden, den_t)
            attn1 = work.tile([R, DIM], F32, tag="attn1")
            nc.vector.scalar_tensor_tensor(
                out=attn1.rearrange("p (h d) -> p h d", h=H),
                in0=o1_ps.rearrange("p (h d) -> p h d", h=H),
                scalar=1.0,
                in1=rden.unsqueeze(2).broadcast_to([R, H, DK]),
                op0=OP.mult, op1=OP.mult)
            a1t_ps = pssm.tile([DIM, R], F32, tag="ps")
            nc.tensor.transpose(a1t_ps, attn1, ident)
            attn1T = work.tile([DIM, R], F32R, tag="attnT")
            nc.vector.tensor_copy(attn1T, a1t_ps)
            fo_ps = pssm.tile([R, DIM], F32, tag="o2", bufs=1)
            nc.tensor.matmul(fo_ps, attn1T, wo_r, start=True, stop=False)

            t2_ps = psbig.tile([P, 32, 4 * H], F32, tag="pb")
            tt_sb = big.tile([E, R * H], F32R)
            tt4 = tt_sb.rearrange("p (g nl h) -> p g nl h", g=32, h=H)
            tt_v = tt_sb.rearrange("p (n h) -> p n h", h=H)
            for half in range(2):
                for g2 in range(half * 16, (half + 1) * 16):
                    for mc in range(NMC):
                        nc.tensor.matmul(
                            t2_ps[:, g2, :],
                            z_sb[:, mc, g2 * 4:(g2 + 1) * 4, :]
                            .rearrange("p n e -> p (n e)"),
                            pt_sb[:, :, mc, g2 * 4:(g2 + 1) * 4],
                            start=(mc == 0), stop=(mc == NMC - 1))
                gs = slice(half * 16, (half + 1) * 16)
                for nl in range(4):
                    nc.vector.tensor_copy(
                        tt4[:, gs, nl, :],
                        t2_ps[nl * 32:nl * 32 + 16, gs, :]
                        .rearrange("p g (h nl2) -> p g h nl2", h=H)[:, :, :, nl])
            o2_ps = pssm.tile([R, DIM], F32, tag="o1h", bufs=1)
            for h in range(H):
                nc.tensor.matmul(o2_ps[:, h * DK:(h + 1) * DK],
                                 tt_v[:, :, h],
                                 wve_r[:, h * DK:(h + 1) * DK],
                                 start=True, stop=True)

            attn2 = work.tile([R, DIM], F32, tag="attn2")
            nc.vector.scalar_tensor_tensor(
                out=attn2.rearrange("p (h d) -> p h d", h=H),
                in0=o2_ps.rearrange("p (h d) -> p h d", h=H),
                scalar=1.0,
                in1=rden.unsqueeze(2).broadcast_to([R, H, DK]),
                op0=OP.mult, op1=OP.mult)
            a2t_ps = pssm.tile([DIM, R], F32, tag="ps")
            nc.tensor.transpose(a2t_ps, attn2, ident)
            attn2T = work.tile([DIM, R], F32R, tag="attnT")
            nc.vector.tensor_copy(attn2T, a2t_ps)
            nc.tensor.matmul(fo_ps, attn2T, wo_r, start=False, stop=True)

            out_sb = work.tile([R, DIM], F32, tag="osb")
            nc.vector.scalar_tensor_tensor(
                out=out_sb, in0=fo_ps, scalar=1.0, in1=boxs_t,
                op0=OP.mult, op1=OP.add)
            nc.sync.dma_start(out=out_d, in_=out_sb)

            if dbg:
                def dout(name, tl):
                    d = nc.dram_tensor(name, list(tl.shape),
                                       tl.dtype, kind="ExternalOutput").ap()
                    nc.sync.dma_start(out=d, in_=tl)
                dout("d_ht", ht_sb)
                dout("d_kt", kt_sb)
                dout("d_qts", qts_sb)
                dout("d_v", v_sb)
                dout("d_bias", bias_sb)
                dout("d_p", p_sb)
                dout("d_den", den_t)
                dout("d_tt", tt_sb)
                dout("d_attn", attn_sb)
                dout("d_pt", pt_sb)
                dout("d_z", z_sb)

    nc.compile()
    return nc


def _get_program():
    global _CACHED
    if _CACHED is None:
        _CACHED = _build_program()
    return _CACHED


def _make_in_maps(inputs):
    x = np.ascontiguousarray(np.asarray(inputs["x"], dtype=np.float32))
    ef = np.asarray(inputs["edge_feat"], dtype=np.float32)
    mask = np.asarray(inputs["mask"])
    Wq = np.asarray(inputs["Wq"], dtype=np.float32)
    Wk = np.asarray(inputs["Wk"], dtype=np.float32)
    Wv = np.asarray(inputs["Wv"], dtype=np.float32)
    Wo = np.ascontiguousarray(np.asarray(inputs["Wo"], dtype=np.float32))
    Wae = np.asarray(inputs["Wae"], dtype=np.float32)
    Wve = np.ascontiguousarray(np.asarray(inputs["Wve"], dtype=np.float32))
    bq = np.asarray(inputs["bq"], dtype=np.float32)
    bv = np.asarray(inputs["bv"], dtype=np.float32)
    bve = np.asarray(inputs.get("bve", np.zeros(DIM)), dtype=np.float32)
    bo = np.asarray(inputs["bo"], dtype=np.float32)
    bae = np.asarray(inputs["bae"], dtype=np.float32)
    gamma = np.asarray(inputs["gamma"], dtype=np.float32)
    beta = np.asarray(inputs["beta"], dtype=np.float32)

    # fold pre-LN gamma/beta into the projections; fold the 1/sqrt(dk)
    # score scale into Wq/bq; fold bv into bo (sum alpha = 1); drop bk
    # (softmax-shift invariant).
    Wq_g = gamma[:, None] * Wq * 0.25
    bq_g = (beta @ Wq + bq) * 0.25
    Wk_g = gamma[:, None] * Wk
    Wv_g = gamma[:, None] * Wv
    bo_g = (beta @ Wv + bv + bve) @ Wo + bo

    # block-diagonal Wae with (h, c) column order:
    # wblk[c*16+e, h*8+c] = Wae[e, h]
    wblk = np.zeros((P, 8 * H), dtype=np.float16)
    for c in range(8):
        for h in range(H):
            wblk[c * E:(c + 1) * E, h * 8 + c] = Wae[:, h].astype(np.float16)

    # mask fold: v0 with Wae^T v0 = NEG * ones -> ef' = ef + (1-mask) v0
    v0, *_ = np.linalg.lstsq(Wae.T, np.full((H,), NEG, np.float32),
                             rcond=None)
    v0 = v0.astype(np.float32)

    # head-padded projection weights/biases: head h -> plane s=h//3,
    # partition group g=h%3 (rows g*32..g*32+16), rest zero.
    def pad_w(W, b):
        Wp = np.zeros((DIM, 3 * DIM), dtype=np.float32)
        bp = np.zeros((DIM, 3), dtype=np.float32)
        for h in range(H):
            s, g = h // 3, h % 3
            Wp[:, s * DIM + g * 32:s * DIM + g * 32 + DK] = \
                W[:, h * DK:(h + 1) * DK]
            bp[g * 32:g * 32 + DK, s] = b[h * DK:(h + 1) * DK]
        return Wp, bp

    Wq_p, bq_p = pad_w(Wq_g, bq_g)
    Wk_p, _ = pad_w(Wk_g, np.zeros(DIM, np.float32))

    wbig = np.ascontiguousarray(
        np.concatenate([Wq_p, Wk_p, Wv_g, Wo], axis=1))

    rep = lambda vec: np.broadcast_to(
        vec.reshape(1, -1), (P, vec.size)).astype(np.float32)
    smalls = np.ascontiguousarray(np.concatenate(
        [bq_p, rep(bae)], axis=1))

    xarr_base = np.ascontiguousarray(
        x.reshape(NMC, P, DIM).transpose(1, 0, 2))   # [P, NMC, DIM]

    shared = {
        "wbig": wbig, "wblk": wblk, "wve": Wve, "smalls": smalls,
    }

    in_maps = []
    for c in range(NCORES):
        rows = slice(c * R, (c + 1) * R)
        ef_slab = ef[rows]                                   # [R, N, E]
        # masked copy for the attention-bias path only
        efb = ef_slab + (~mask[rows])[:, :, None].astype(np.float32) \
            * v0[None, None, :]
        # X[(c,e), j, n] = efb[n, j*8+c, e], contiguous fp16
        efx = np.ascontiguousarray(
            efb.reshape(R, 128, 8, E).transpose(2, 3, 1, 0)
        ).reshape(P, 128 * P).astype(np.float16)
        efm_t = ef_slab.transpose(1, 0, 2).astype(ml_dtypes.float8_e4m3)
        efm = np.zeros((N, R, 32), dtype=ml_dtypes.float8_e4m3)
        efm[:, :, 0:E] = efm_t
        efm = efm.reshape(N, R * 32)
        xarr = np.concatenate(
            [xarr_base, x[rows][:, None, :]], axis=1).reshape(P, -1)
        in_maps.append({
            **shared,
            "efx": efx, "efm": efm,
            "xarr": np.ascontiguousarray(xarr),
            "boxs": np.ascontiguousarray(x[rows] + bo_g[None, :]),
        })
    return in_maps


def kernel(**inputs) -> np.ndarray:
    in_maps = _make_in_maps(inputs)
    nc = _get_program()
    res = run_bass_kernel_spmd(nc, in_maps, list(range(NCORES)))
    out = np.concatenate([res.results[c]["out"] for c in range(NCORES)],
                         axis=0)
    return out.astype(np.float32)
